# revision 3
# baseline (speedup 1.0000x reference)
"""Trainium2 Bass kernel v4 for nn_EquivariantLocalScoreMachine.

Math: for each query pixel q (B*H*W=4096, 27-dim circular 3x3 patch xq) over
dataset patches p (N*H*W=32768, zero-padded 3x3 patches):
    log_w[q,p] = b[q] + a*<xq, patch_p> + c[p],   a = mu/sigma^2
(b[q] cancels in the final ratio).  Output:
    out[q,ch] = (mu * wc[q,ch]/sum_w[q] - x[q,ch]) / sigma^2

Measured PE behavior drives the v4 structure: back-to-back SAME-shape
512-col matmuls sustain ~220ns each, but every lhsT SHAPE switch costs ~90ns
(ldweights pipe reconfig), and the Tile scheduler freely interleaves score
and acc matmuls.  So v4 makes EVERY matmul the identical shape
[128,128] x [128,512] -> [128,512]:
  * scores: K padded 111->128 (zero rows), one matmul per (chunk, q-tile).
  * acc: lhsT is a zero-padded [128,128] pw block whose only nonzero columns
    (pc0,pc1,pc2,1) sit at column offset 32*(t%4); the matmul writes the
    FULL [128,512] accumulator bank, adding zeros everywhere except tile
    t's 4 stat rows.  4 tiles pack per bank at partition bases 0/32/64/96,
    so 8 q-tile accumulators use only 2 PSUM banks.
  * per-q stabilizer shift (subset max + margin, cancels exactly) is
    computed on the host into row 0 of xa; no device subset pass.
  * 6 remaining PSUM banks form 3 double-width score slots [128,1024]; exp
    runs once per slot: scalar Exp (bias=-B/A) or DVE Schraudolph
    (uint16 = sat(round(A*psA)) == bf16 bits of e^(psA-B/A)), split ~54/46.

Sharding: patch axis split across 8 cores (4096 each); host merges the
per-core (sum_w, wc) partials with an exact logsumexp rescale in fp64.
"""
import sys
import numpy as np

for _p in ("/opt/trn_rl_repo", "/opt/pypackages"):
    if _p not in sys.path:
        sys.path.append(_p)

import ml_dtypes

BF16 = ml_dtypes.bfloat16

B, C, H, W = 4, 3, 32, 32
N_IMG = 32
NQ = B * H * W            # 4096 queries
NP = N_IMG * H * W        # 32768 dataset patches
NCORES = 8
PLOC = NP // NCORES       # 4096 patches per core
NCHUNK = PLOC // 128      # 32 p-chunks per core
NT = NQ // 512            # 8 q-tiles
SUB_STRIDE = 16
MARGIN = 40.0
KA = 128                  # padded rows: shift + 4*27 cross + c_hi + c_lo + 0s
A_S = 128.0 / np.log(2.0)
B_S = 16256.0 - 128.0 * (np.log2(1.08606) / 2.0)
BIAS = -B_S / A_S

_prog_cache = {}


def _build_program():
    if "nc" in _prog_cache:
        return _prog_cache["nc"]
    from contextlib import ExitStack
    import concourse.bass as bass
    import concourse.tile as tile
    from concourse import mybir

    f32 = mybir.dt.float32
    bf = mybir.dt.bfloat16
    u16 = mybir.dt.uint16
    nc = bass.Bass("TRN2", num_devices=NCORES, debug=False)
    patm_d = nc.dram_tensor("patm", [KA, PLOC], bf, kind="ExternalInput").ap()
    xa_d = nc.dram_tensor("xa", [KA, NQ], bf, kind="ExternalInput").ap()
    # pw variant k in [0,4): [128, NCHUNK, 128] with the real 4 cols of each
    # chunk at column offset 32k
    pw_d = [nc.dram_tensor(f"pw{k}", [128, NCHUNK, 128], bf,
                           kind="ExternalInput").ap() for k in range(4)]
    out_d = nc.dram_tensor("out", [4, NQ], f32, kind="ExternalOutput").ap()

    with tile.TileContext(nc) as tc, ExitStack() as ctx:
        consts = ctx.enter_context(tc.tile_pool(name="consts", bufs=1))
        ps_pool = ctx.enter_context(tc.tile_pool(name="ps", bufs=1, space="PSUM"))
        ps_big = ctx.enter_context(tc.tile_pool(name="ps_big", bufs=3, space="PSUM"))
        wpool = ctx.enter_context(tc.tile_pool(name="wpool", bufs=6))
        dpool = ctx.enter_context(tc.tile_pool(name="dpool", bufs=2))

        # DMA order = first use: patm chunks 0-7, xa (4 slices), pw windows,
        # then the rest
        s_patm = consts.tile([KA, PLOC], bf)
        s_xa = consts.tile([KA, NQ], bf)
        s_pw = [consts.tile([128, NCHUNK, 128], bf, name=f"s_pw{k}")
                for k in range(4)]
        # fine-grained head so the first units start within ~2us
        nc.sync.dma_start(out=s_patm[:, 0:256], in_=patm_d[:, 0:256])
        nc.sync.dma_start(out=s_xa[:, 0:1024], in_=xa_d[:, 0:1024])
        nc.sync.dma_start(out=s_xa[:, 1024:2048], in_=xa_d[:, 1024:2048])
        for k in range(4):
            nc.sync.dma_start(out=s_pw[k][:, 0:2, :], in_=pw_d[k][:, 0:2, :])
        nc.sync.dma_start(out=s_xa[:, 2048:3072], in_=xa_d[:, 2048:3072])
        nc.sync.dma_start(out=s_xa[:, 3072:4096], in_=xa_d[:, 3072:4096])
        nc.sync.dma_start(out=s_patm[:, 256:1024], in_=patm_d[:, 256:1024])
        for k in range(4):
            nc.sync.dma_start(out=s_pw[k][:, 2:8, :], in_=pw_d[k][:, 2:8, :])
        nc.sync.dma_start(out=s_patm[:, 1024:2048], in_=patm_d[:, 1024:2048])
        for k in range(4):
            nc.sync.dma_start(out=s_pw[k][:, 8:16, :], in_=pw_d[k][:, 8:16, :])
        nc.sync.dma_start(out=s_patm[:, 2048:], in_=patm_d[:, 2048:])
        for k in range(4):
            nc.sync.dma_start(out=s_pw[k][:, 16:, :], in_=pw_d[k][:, 16:, :])

        s_bias = consts.tile([128, 1], f32)
        nc.vector.memset(s_bias, BIAS)

        # 3 double-buffered [128,1024] score slots (6 banks) + 2 acc banks
        accb = [ps_pool.tile([128, 512], f32, name=f"accb{b}") for b in range(2)]

        wgts = {}
        psAs = {}

        def sc_unit(ch, u):
            g = ch * 4 + u
            psA = ps_big.tile([128, 1024], f32, tag="big", name="psA")
            psAs[g] = psA
            for i in range(2):
                t = 2 * u + i
                nc.tensor.matmul(
                    psA[:, i * 512:(i + 1) * 512],
                    s_patm[:, ch * 128:(ch + 1) * 128],
                    s_xa[:, t * 512:(t + 1) * 512],
                    start=True, stop=True)
            wgt = wpool.tile([128, 1024], bf, name="wgt")
            wgts[g] = wgt
            psA = psAs.pop(g)
            if g % 13 < 7:
                nc.scalar.activation(wgt, psA,
                                     mybir.ActivationFunctionType.Exp,
                                     bias=s_bias[:, 0:1])
            else:
                nc.vector.tensor_scalar(
                    wgt[:, :].bitcast(u16), psA, A_S, 0.0,
                    mybir.AluOpType.mult, mybir.AluOpType.max)

        def acc_unit(ch, u):
            g = ch * 4 + u
            wgt = wgts.pop(g)
            for i in range(2):
                t = 2 * u + i
                nc.tensor.matmul(
                    accb[t // 4], s_pw[t % 4][:, ch, :],
                    wgt[:, i * 512:(i + 1) * 512],
                    start=(ch == 0 and t % 4 == 0),
                    stop=(ch == NCHUNK - 1 and t % 4 == 3),
                    skip_group_check=True)

        # steady interleave: acc trails its score unit by 2 units, so the
        # exp (~1.2us) always completes before the acc is considered
        units = [(ch, u) for ch in range(NCHUNK) for u in range(4)]
        for gi, (ch, u) in enumerate(units):
            sc_unit(ch, u)
            if gi >= 2:
                acc_unit(*units[gi - 2])
        acc_unit(*units[-2])
        acc_unit(*units[-1])

        for b in range(2):
            dr = dpool.tile([128, 512], f32, name="dr")
            nc.scalar.copy(dr, accb[b])
            for k in range(4):
                t = 4 * b + k
                nc.sync.dma_start(out=out_d[:, t * 512:(t + 1) * 512],
                                  in_=dr[32 * k:32 * k + 4, :])

    # single-wait post-pass
    for blk in nc.m.functions[0].blocks:
        out, changed = [], False
        for ins in blk.instructions:
            si = ins.sync_info
            w = list(si.on_wait) if si is not None and si.on_wait else []
            if len(w) > 1:
                for k, extra in enumerate(w[:-1]):
                    out.append(mybir.InstNoOp(
                        name=f"{ins.name}_sw{k}", engine=ins.engine,
                        sync_info=mybir.SyncInfo(on_wait=[extra], on_update=[]),
                        bass_nofuse=True))
                ins.sync_info = mybir.SyncInfo(
                    on_wait=[w[-1]], on_update=list(si.on_update))
                changed = True
            out.append(ins)
        if changed:
            blk.instructions = out

    _prog_cache["nc"] = nc
    return nc


def _im2col(arr, wrap):
    if wrap:
        ap = np.pad(arr, ((0, 0), (0, 0), (1, 1), (1, 1)), mode="wrap")
    else:
        ap = np.pad(arr, ((0, 0), (0, 0), (1, 1), (1, 1)))
    sl = [ap[:, :, di:di + H, dj:dj + W] for di in range(3) for dj in range(3)]
    st = np.stack(sl, axis=2)
    return st.transpose(0, 3, 4, 1, 2).reshape(arr.shape[0] * H * W, C * 9)


def _split(v):
    h = v.astype(BF16)
    l = (v - h.astype(np.float32)).astype(BF16)
    return h, l


def _run(inputs, trace=False):
    from concourse.bass_utils import run_bass_kernel_spmd

    x = np.ascontiguousarray(np.asarray(inputs["x"], dtype=np.float32))
    images = np.ascontiguousarray(np.asarray(inputs["images"], dtype=np.float32))
    t = int(np.asarray(inputs["t"]))
    mu = float(np.asarray(inputs["mu_sched"])[t])
    sigma = float(np.asarray(inputs["sigma_sched"])[t])
    a = mu / (sigma * sigma)
    inv2s2 = 1.0 / (2.0 * sigma * sigma)

    xq = _im2col(x, wrap=True)
    patches = _im2col(images, wrap=False)
    c_all = ((-mu * mu * inv2s2) * np.sum(patches * patches, axis=1)).astype(np.float32)
    pcent = patches.reshape(NP, C, 9)[:, :, 4]

    v = (a * xq).astype(np.float32)
    xh, xl = _split(v)
    xhf = xh.astype(np.float32)

    xa_base = np.zeros((KA, NQ), BF16)
    xa_base[1:28] = xh.T
    xa_base[28:55] = xh.T
    xa_base[55:82] = xl.T
    xa_base[82:109] = xl.T
    xa_base[109] = BF16(1.0)
    xa_base[110] = BF16(1.0)

    in_maps = []
    zs = []
    for cc in range(NCORES):
        lo = cc * PLOC
        pf = patches[lo:lo + PLOC].astype(np.float32)
        ph, pl = _split(pf)
        ch_, cl = _split(c_all[lo:lo + PLOC])
        patm = np.zeros((KA, PLOC), BF16)
        patm[0] = BF16(1.0)
        patm[1:28] = ph.T
        patm[28:55] = pl.T
        patm[55:82] = ph.T
        patm[82:109] = pl.T
        patm[109] = ch_
        patm[110] = cl

        sub = slice(lo, lo + PLOC, SUB_STRIDE)
        gsub = (xhf @ patches[sub].T.astype(np.float32)
                + c_all[sub][None, :]).max(axis=1)
        zrow = (-(gsub + MARGIN) + B_S / A_S).astype(BF16)
        zs.append(zrow.astype(np.float32))

        xa = xa_base.copy()
        xa[0] = zrow

        pcb = pcent[lo:lo + PLOC].astype(BF16)
        pws = []
        for k in range(4):
            pw = np.zeros((128, NCHUNK, 128), BF16)
            for chnk in range(NCHUNK):
                pw[:, chnk, 32 * k:32 * k + 3] = pcb[chnk * 128:(chnk + 1) * 128]
                pw[:, chnk, 32 * k + 3] = BF16(1.0)
            pws.append(pw)
        m = {"patm": patm, "xa": xa}
        for k in range(4):
            m[f"pw{k}"] = pws[k]
        in_maps.append(m)

    nc = _build_program()
    res = run_bass_kernel_spmd(nc, in_maps, core_ids=list(range(NCORES)),
                               trace=trace)

    # host merge: core weights = e^{g + z_c - B/A}; rescale to min z frame
    z = np.stack(zs)                                   # [8, NQ]
    part = np.stack([r["out"] for r in res.results])   # [8, 4, NQ]
    Z = z.min(axis=0)
    fac = np.exp((Z[None, :] - z).astype(np.float64))  # [8, NQ] <= 1
    sum_w = (part[:, 3].astype(np.float64) * fac).sum(axis=0)
    wc = (part[:, 0:3].astype(np.float64) * fac[:, None, :]).sum(axis=0)

    xcT = x.reshape(B, C, H * W).transpose(1, 0, 2).reshape(C, NQ)
    out_q = (mu * wc / sum_w[None, :] - xcT) / (sigma * sigma)
    out = out_q.reshape(C, B, H, W).transpose(1, 0, 2, 3).astype(np.float32)
    return out, res


def kernel(**inputs) -> np.ndarray:
    out, _ = _run(inputs, trace=False)
    return out


# revision 4
# speedup vs baseline: 1.0002x; 1.0002x over previous
"""Trainium2 Bass kernel v4 for nn_EquivariantLocalScoreMachine.

Math: for each query pixel q (B*H*W=4096, 27-dim circular 3x3 patch xq) over
dataset patches p (N*H*W=32768, zero-padded 3x3 patches):
    log_w[q,p] = b[q] + a*<xq, patch_p> + c[p],   a = mu/sigma^2
(b[q] cancels in the final ratio).  Output:
    out[q,ch] = (mu * wc[q,ch]/sum_w[q] - x[q,ch]) / sigma^2

Measured PE behavior drives the v4 structure: back-to-back SAME-shape
512-col matmuls sustain ~220ns each, but every lhsT SHAPE switch costs ~90ns
(ldweights pipe reconfig), and the Tile scheduler freely interleaves score
and acc matmuls.  So v4 makes EVERY matmul the identical shape
[128,128] x [128,512] -> [128,512]:
  * scores: K padded 111->128 (zero rows), one matmul per (chunk, q-tile).
  * acc: lhsT is a zero-padded [128,128] pw block whose only nonzero columns
    (pc0,pc1,pc2,1) sit at column offset 32*(t%4); the matmul writes the
    FULL [128,512] accumulator bank, adding zeros everywhere except tile
    t's 4 stat rows.  4 tiles pack per bank at partition bases 0/32/64/96,
    so 8 q-tile accumulators use only 2 PSUM banks.
  * per-q stabilizer shift (subset max + margin, cancels exactly) is
    computed on the host into row 0 of xa; no device subset pass.
  * 6 remaining PSUM banks form 3 double-width score slots [128,1024]; exp
    runs once per slot: scalar Exp (bias=-B/A) or DVE Schraudolph
    (uint16 = sat(round(A*psA)) == bf16 bits of e^(psA-B/A)), split ~54/46.

Sharding: patch axis split across 8 cores (4096 each); host merges the
per-core (sum_w, wc) partials with an exact logsumexp rescale in fp64.
"""
import sys
import numpy as np

for _p in ("/opt/trn_rl_repo", "/opt/pypackages"):
    if _p not in sys.path:
        sys.path.append(_p)

import ml_dtypes

BF16 = ml_dtypes.bfloat16

B, C, H, W = 4, 3, 32, 32
N_IMG = 32
NQ = B * H * W            # 4096 queries
NP = N_IMG * H * W        # 32768 dataset patches
NCORES = 8
PLOC = NP // NCORES       # 4096 patches per core
NCHUNK = PLOC // 128      # 32 p-chunks per core
NT = NQ // 512            # 8 q-tiles
SUB_STRIDE = 16
MARGIN = 40.0
KA = 128                  # padded rows: shift + 4*27 cross + c_hi + c_lo + 0s
A_S = 128.0 / np.log(2.0)
B_S = 16256.0 - 128.0 * (np.log2(1.08606) / 2.0)
BIAS = -B_S / A_S

_prog_cache = {}


def _build_program():
    if "nc" in _prog_cache:
        return _prog_cache["nc"]
    from contextlib import ExitStack
    import concourse.bass as bass
    import concourse.tile as tile
    from concourse import mybir

    f32 = mybir.dt.float32
    bf = mybir.dt.bfloat16
    u16 = mybir.dt.uint16
    nc = bass.Bass("TRN2", num_devices=NCORES, debug=False)
    patm_d = nc.dram_tensor("patm", [KA, PLOC], bf, kind="ExternalInput").ap()
    xa_d = nc.dram_tensor("xa", [KA, NQ], bf, kind="ExternalInput").ap()
    # pw variant k in [0,4): [128, NCHUNK, 128] with the real 4 cols of each
    # chunk at column offset 32k
    pw_d = [nc.dram_tensor(f"pw{k}", [128, NCHUNK, 128], bf,
                           kind="ExternalInput").ap() for k in range(4)]
    out_d = nc.dram_tensor("out", [4, NQ], f32, kind="ExternalOutput").ap()

    with tile.TileContext(nc) as tc, ExitStack() as ctx:
        consts = ctx.enter_context(tc.tile_pool(name="consts", bufs=1))
        ps_pool = ctx.enter_context(tc.tile_pool(name="ps", bufs=1, space="PSUM"))
        ps_big = ctx.enter_context(tc.tile_pool(name="ps_big", bufs=3, space="PSUM"))
        wpool = ctx.enter_context(tc.tile_pool(name="wpool", bufs=6))
        dpool = ctx.enter_context(tc.tile_pool(name="dpool", bufs=2))

        # DMA order = first use: patm chunks 0-7, xa (4 slices), pw windows,
        # then the rest
        s_patm = consts.tile([KA, PLOC], bf)
        s_xa = consts.tile([KA, NQ], bf)
        s_pw = [consts.tile([128, NCHUNK, 128], bf, name=f"s_pw{k}")
                for k in range(4)]
        # fine-grained head so the first units start within ~2us
        nc.sync.dma_start(out=s_patm[:, 0:256], in_=patm_d[:, 0:256])
        nc.sync.dma_start(out=s_xa[:, 0:1024], in_=xa_d[:, 0:1024])
        nc.sync.dma_start(out=s_xa[:, 1024:2048], in_=xa_d[:, 1024:2048])
        for k in range(4):
            nc.sync.dma_start(out=s_pw[k][:, 0:2, :], in_=pw_d[k][:, 0:2, :])
        nc.sync.dma_start(out=s_xa[:, 2048:3072], in_=xa_d[:, 2048:3072])
        nc.sync.dma_start(out=s_xa[:, 3072:4096], in_=xa_d[:, 3072:4096])
        nc.sync.dma_start(out=s_patm[:, 256:1024], in_=patm_d[:, 256:1024])
        for k in range(4):
            nc.sync.dma_start(out=s_pw[k][:, 2:8, :], in_=pw_d[k][:, 2:8, :])
        nc.sync.dma_start(out=s_patm[:, 1024:2048], in_=patm_d[:, 1024:2048])
        for k in range(4):
            nc.sync.dma_start(out=s_pw[k][:, 8:16, :], in_=pw_d[k][:, 8:16, :])
        nc.sync.dma_start(out=s_patm[:, 2048:], in_=patm_d[:, 2048:])
        for k in range(4):
            nc.sync.dma_start(out=s_pw[k][:, 16:, :], in_=pw_d[k][:, 16:, :])

        s_bias = consts.tile([128, 1], f32)
        nc.vector.memset(s_bias, BIAS)

        # 3 double-buffered [128,1024] score slots (6 banks) + 2 acc banks
        accb = [ps_pool.tile([128, 512], f32, name=f"accb{b}") for b in range(2)]

        wgts = {}
        psAs = {}

        def sc_unit(ch, u):
            g = ch * 4 + u
            psA = ps_big.tile([128, 1024], f32, tag="big", name="psA")
            psAs[g] = psA
            for i in range(2):
                t = 2 * u + i
                nc.tensor.matmul(
                    psA[:, i * 512:(i + 1) * 512],
                    s_patm[:, ch * 128:(ch + 1) * 128],
                    s_xa[:, t * 512:(t + 1) * 512],
                    start=True, stop=True)
            wgt = wpool.tile([128, 1024], bf, name="wgt")
            wgts[g] = wgt
            psA = psAs.pop(g)
            if g % 13 < 7:
                nc.scalar.activation(wgt, psA,
                                     mybir.ActivationFunctionType.Exp,
                                     bias=s_bias[:, 0:1])
            else:
                nc.vector.tensor_scalar(
                    wgt[:, :].bitcast(u16), psA, A_S, 0.0,
                    mybir.AluOpType.mult, mybir.AluOpType.max)

        def acc_unit(ch, u):
            g = ch * 4 + u
            wgt = wgts.pop(g)
            for i in range(2):
                t = 2 * u + i
                nc.tensor.matmul(
                    accb[t // 4], s_pw[t % 4][:, ch, :],
                    wgt[:, i * 512:(i + 1) * 512],
                    start=(ch == 0 and t % 4 == 0),
                    stop=(ch == NCHUNK - 1 and t % 4 == 3),
                    skip_group_check=True)

        # steady interleave: acc trails its score unit by 2 units, so the
        # exp (~1.2us) always completes before the acc is considered
        units = [(ch, u) for ch in range(NCHUNK) for u in range(4)]
        for gi, (ch, u) in enumerate(units):
            sc_unit(ch, u)
            if gi >= 3:
                acc_unit(*units[gi - 3])
        for gg in (-3, -2, -1):
            acc_unit(*units[gg])

        for b in range(2):
            dr = dpool.tile([128, 512], f32, name="dr")
            nc.scalar.copy(dr, accb[b])
            for k in range(4):
                t = 4 * b + k
                nc.sync.dma_start(out=out_d[:, t * 512:(t + 1) * 512],
                                  in_=dr[32 * k:32 * k + 4, :])

    # single-wait post-pass
    for blk in nc.m.functions[0].blocks:
        out, changed = [], False
        for ins in blk.instructions:
            si = ins.sync_info
            w = list(si.on_wait) if si is not None and si.on_wait else []
            if len(w) > 1:
                for k, extra in enumerate(w[:-1]):
                    out.append(mybir.InstNoOp(
                        name=f"{ins.name}_sw{k}", engine=ins.engine,
                        sync_info=mybir.SyncInfo(on_wait=[extra], on_update=[]),
                        bass_nofuse=True))
                ins.sync_info = mybir.SyncInfo(
                    on_wait=[w[-1]], on_update=list(si.on_update))
                changed = True
            out.append(ins)
        if changed:
            blk.instructions = out

    _prog_cache["nc"] = nc
    return nc


def _im2col(arr, wrap):
    if wrap:
        ap = np.pad(arr, ((0, 0), (0, 0), (1, 1), (1, 1)), mode="wrap")
    else:
        ap = np.pad(arr, ((0, 0), (0, 0), (1, 1), (1, 1)))
    sl = [ap[:, :, di:di + H, dj:dj + W] for di in range(3) for dj in range(3)]
    st = np.stack(sl, axis=2)
    return st.transpose(0, 3, 4, 1, 2).reshape(arr.shape[0] * H * W, C * 9)


def _split(v):
    h = v.astype(BF16)
    l = (v - h.astype(np.float32)).astype(BF16)
    return h, l


def _run(inputs, trace=False):
    from concourse.bass_utils import run_bass_kernel_spmd

    x = np.ascontiguousarray(np.asarray(inputs["x"], dtype=np.float32))
    images = np.ascontiguousarray(np.asarray(inputs["images"], dtype=np.float32))
    t = int(np.asarray(inputs["t"]))
    mu = float(np.asarray(inputs["mu_sched"])[t])
    sigma = float(np.asarray(inputs["sigma_sched"])[t])
    a = mu / (sigma * sigma)
    inv2s2 = 1.0 / (2.0 * sigma * sigma)

    xq = _im2col(x, wrap=True)
    patches = _im2col(images, wrap=False)
    c_all = ((-mu * mu * inv2s2) * np.sum(patches * patches, axis=1)).astype(np.float32)
    pcent = patches.reshape(NP, C, 9)[:, :, 4]

    v = (a * xq).astype(np.float32)
    xh, xl = _split(v)
    xhf = xh.astype(np.float32)

    xa_base = np.zeros((KA, NQ), BF16)
    xa_base[1:28] = xh.T
    xa_base[28:55] = xh.T
    xa_base[55:82] = xl.T
    xa_base[82:109] = xl.T
    xa_base[109] = BF16(1.0)
    xa_base[110] = BF16(1.0)

    in_maps = []
    zs = []
    for cc in range(NCORES):
        lo = cc * PLOC
        pf = patches[lo:lo + PLOC].astype(np.float32)
        ph, pl = _split(pf)
        ch_, cl = _split(c_all[lo:lo + PLOC])
        patm = np.zeros((KA, PLOC), BF16)
        patm[0] = BF16(1.0)
        patm[1:28] = ph.T
        patm[28:55] = pl.T
        patm[55:82] = ph.T
        patm[82:109] = pl.T
        patm[109] = ch_
        patm[110] = cl

        sub = slice(lo, lo + PLOC, SUB_STRIDE)
        gsub = (xhf @ patches[sub].T.astype(np.float32)
                + c_all[sub][None, :]).max(axis=1)
        zrow = (-(gsub + MARGIN) + B_S / A_S).astype(BF16)
        zs.append(zrow.astype(np.float32))

        xa = xa_base.copy()
        xa[0] = zrow

        pcb = pcent[lo:lo + PLOC].astype(BF16)
        pws = []
        for k in range(4):
            pw = np.zeros((128, NCHUNK, 128), BF16)
            for chnk in range(NCHUNK):
                pw[:, chnk, 32 * k:32 * k + 3] = pcb[chnk * 128:(chnk + 1) * 128]
                pw[:, chnk, 32 * k + 3] = BF16(1.0)
            pws.append(pw)
        m = {"patm": patm, "xa": xa}
        for k in range(4):
            m[f"pw{k}"] = pws[k]
        in_maps.append(m)

    nc = _build_program()
    res = run_bass_kernel_spmd(nc, in_maps, core_ids=list(range(NCORES)),
                               trace=trace)

    # host merge: core weights = e^{g + z_c - B/A}; rescale to min z frame
    z = np.stack(zs)                                   # [8, NQ]
    part = np.stack([r["out"] for r in res.results])   # [8, 4, NQ]
    Z = z.min(axis=0)
    fac = np.exp((Z[None, :] - z).astype(np.float64))  # [8, NQ] <= 1
    sum_w = (part[:, 3].astype(np.float64) * fac).sum(axis=0)
    wc = (part[:, 0:3].astype(np.float64) * fac[:, None, :]).sum(axis=0)

    xcT = x.reshape(B, C, H * W).transpose(1, 0, 2).reshape(C, NQ)
    out_q = (mu * wc / sum_w[None, :] - xcT) / (sigma * sigma)
    out = out_q.reshape(C, B, H, W).transpose(1, 0, 2, 3).astype(np.float32)
    return out, res


def kernel(**inputs) -> np.ndarray:
    out, _ = _run(inputs, trace=False)
    return out


# revision 5
# speedup vs baseline: 1.0027x; 1.0025x over previous
"""Trainium2 Bass kernel v4 for nn_EquivariantLocalScoreMachine.

Math: for each query pixel q (B*H*W=4096, 27-dim circular 3x3 patch xq) over
dataset patches p (N*H*W=32768, zero-padded 3x3 patches):
    log_w[q,p] = b[q] + a*<xq, patch_p> + c[p],   a = mu/sigma^2
(b[q] cancels in the final ratio).  Output:
    out[q,ch] = (mu * wc[q,ch]/sum_w[q] - x[q,ch]) / sigma^2

Measured PE behavior drives the structure: back-to-back SAME-shape 512-col
matmuls sustain ~220ns each, but every lhsT SHAPE switch costs ~90ns
(ldweights pipe reconfig), and the Tile scheduler freely interleaves score
and acc matmuls.  So EVERY matmul has the identical shape
[128,128] x [128,512] -> [128,512]:
  * scores: K padded 111->128 (zero rows), one matmul per (chunk, q-tile).
  * acc: lhsT is a zero-padded [128,128] pw block whose only nonzero columns
    (pc0,pc1,pc2,1) sit at column offset 32*(t%4); the matmul writes the
    FULL [128,512] accumulator bank, adding zeros everywhere except tile
    t's 4 stat rows.  4 tiles pack per bank at partition bases 0/32/64/96,
    so 8 q-tile accumulators use only 2 PSUM banks.
  * per-q stabilizer shift (subset max + margin, cancels exactly) is
    computed on the host into row 0 of xa; no device subset pass.
  * 6 remaining PSUM banks form 3 double-width score slots [128,1024]; exp
    runs once per slot: scalar Exp (bias=-B/A) or DVE Schraudolph
    (uint16 = sat(round(A*psA)) == bf16 bits of e^(psA-B/A)), split 7/13.
  * emission order: each unit's acc trails its score unit by 3 units, so
    the ~1.2us exp latency is always absorbed and the PE never stalls; the
    scheduler then sustains ~235 ns/matmul (~the measured mixed-pair floor).

Sharding: patch axis split across 8 cores (4096 each); host merges the
per-core (sum_w, wc) partials with an exact logsumexp rescale in fp64.
"""
import sys
import numpy as np

for _p in ("/opt/trn_rl_repo", "/opt/pypackages"):
    if _p not in sys.path:
        sys.path.append(_p)

import ml_dtypes

BF16 = ml_dtypes.bfloat16

B, C, H, W = 4, 3, 32, 32
N_IMG = 32
NQ = B * H * W            # 4096 queries
NP = N_IMG * H * W        # 32768 dataset patches
NCORES = 8
PLOC = NP // NCORES       # 4096 patches per core
NCHUNK = PLOC // 128      # 32 p-chunks per core
NT = NQ // 512            # 8 q-tiles
SUB_STRIDE = 16
MARGIN = 40.0
KA = 128                  # padded rows: shift + 4*27 cross + c_hi + c_lo + 0s
A_S = 128.0 / np.log(2.0)
B_S = 16256.0 - 128.0 * (np.log2(1.08606) / 2.0)
BIAS = -B_S / A_S

_prog_cache = {}


def _build_program():
    if "nc" in _prog_cache:
        return _prog_cache["nc"]
    from contextlib import ExitStack
    import concourse.bass as bass
    import concourse.tile as tile
    from concourse import mybir

    f32 = mybir.dt.float32
    bf = mybir.dt.bfloat16
    u16 = mybir.dt.uint16
    nc = bass.Bass("TRN2", num_devices=NCORES, debug=False)
    patm_d = nc.dram_tensor("patm", [KA, PLOC], bf, kind="ExternalInput").ap()
    xa_d = nc.dram_tensor("xa", [KA, NQ], bf, kind="ExternalInput").ap()
    # pw variant k in [0,4): [128, NCHUNK, 128] with the real 4 cols of each
    # chunk at column offset 32k
    pw_d = [nc.dram_tensor(f"pw{k}", [128, NCHUNK, 128], bf,
                           kind="ExternalInput").ap() for k in range(4)]
    out_d = nc.dram_tensor("out", [4, NQ], f32, kind="ExternalOutput").ap()

    with tile.TileContext(nc) as tc, ExitStack() as ctx:
        consts = ctx.enter_context(tc.tile_pool(name="consts", bufs=1))
        ps_pool = ctx.enter_context(tc.tile_pool(name="ps", bufs=1, space="PSUM"))
        ps_big = ctx.enter_context(tc.tile_pool(name="ps_big", bufs=3, space="PSUM"))
        wpool = ctx.enter_context(tc.tile_pool(name="wpool", bufs=6))
        dpool = ctx.enter_context(tc.tile_pool(name="dpool", bufs=2))

        # DMA order = first use: patm chunks 0-7, xa (4 slices), pw windows,
        # then the rest
        s_patm = consts.tile([KA, PLOC], bf)
        s_xa = consts.tile([KA, NQ], bf)
        s_pw = [consts.tile([128, NCHUNK, 128], bf, name=f"s_pw{k}")
                for k in range(4)]
        # fine-grained head so the first units start within ~2us
        nc.sync.dma_start(out=s_patm[:, 0:256], in_=patm_d[:, 0:256])
        nc.sync.dma_start(out=s_xa[:, 0:1024], in_=xa_d[:, 0:1024])
        nc.sync.dma_start(out=s_xa[:, 1024:2048], in_=xa_d[:, 1024:2048])
        for k in range(4):
            nc.sync.dma_start(out=s_pw[k][:, 0:2, :], in_=pw_d[k][:, 0:2, :])
        nc.sync.dma_start(out=s_xa[:, 2048:3072], in_=xa_d[:, 2048:3072])
        nc.sync.dma_start(out=s_xa[:, 3072:4096], in_=xa_d[:, 3072:4096])
        nc.sync.dma_start(out=s_patm[:, 256:1024], in_=patm_d[:, 256:1024])
        for k in range(4):
            nc.sync.dma_start(out=s_pw[k][:, 2:8, :], in_=pw_d[k][:, 2:8, :])
        nc.sync.dma_start(out=s_patm[:, 1024:2048], in_=patm_d[:, 1024:2048])
        for k in range(4):
            nc.sync.dma_start(out=s_pw[k][:, 8:16, :], in_=pw_d[k][:, 8:16, :])
        nc.sync.dma_start(out=s_patm[:, 2048:], in_=patm_d[:, 2048:])
        for k in range(4):
            nc.sync.dma_start(out=s_pw[k][:, 16:, :], in_=pw_d[k][:, 16:, :])

        s_bias = consts.tile([128, 1], f32)
        nc.vector.memset(s_bias, BIAS)

        # 3 double-buffered [128,1024] score slots (6 banks) + 2 acc banks
        accb = [ps_pool.tile([128, 512], f32, name=f"accb{b}") for b in range(2)]

        wgts = {}
        psAs = {}

        def sc_unit(ch, u):
            g = ch * 4 + u
            psA = ps_big.tile([128, 1024], f32, tag="big", name="psA")
            psAs[g] = psA
            for i in range(2):
                t = 2 * u + i
                nc.tensor.matmul(
                    psA[:, i * 512:(i + 1) * 512],
                    s_patm[:, ch * 128:(ch + 1) * 128],
                    s_xa[:, t * 512:(t + 1) * 512],
                    start=True, stop=True)
            wgt = wpool.tile([128, 1024], bf, name="wgt")
            wgts[g] = wgt
            psA = psAs.pop(g)
            if g % 13 < 7:
                nc.scalar.activation(wgt, psA,
                                     mybir.ActivationFunctionType.Exp,
                                     bias=s_bias[:, 0:1])
            else:
                nc.vector.tensor_scalar(
                    wgt[:, :].bitcast(u16), psA, A_S, 0.0,
                    mybir.AluOpType.mult, mybir.AluOpType.max)

        def acc_unit(ch, u):
            g = ch * 4 + u
            wgt = wgts.pop(g)
            for i in range(2):
                t = 2 * u + i
                nc.tensor.matmul(
                    accb[t // 4], s_pw[t % 4][:, ch, :],
                    wgt[:, i * 512:(i + 1) * 512],
                    start=(ch == 0 and t % 4 == 0),
                    stop=(ch == NCHUNK - 1 and t % 4 == 3),
                    skip_group_check=True)

        # steady interleave: acc trails its score unit by 2 units, so the
        # exp (~1.2us) always completes before the acc is considered
        units = [(ch, u) for ch in range(NCHUNK) for u in range(4)]
        for gi, (ch, u) in enumerate(units):
            sc_unit(ch, u)
            if gi >= 3:
                acc_unit(*units[gi - 3])
        for gg in (-3, -2, -1):
            acc_unit(*units[gg])

        for b in range(2):
            dr = dpool.tile([128, 512], f32, name="dr")
            nc.scalar.copy(dr, accb[b])
            for k in range(4):
                t = 4 * b + k
                nc.sync.dma_start(out=out_d[:, t * 512:(t + 1) * 512],
                                  in_=dr[32 * k:32 * k + 4, :])

    # single-wait post-pass
    for blk in nc.m.functions[0].blocks:
        out, changed = [], False
        for ins in blk.instructions:
            si = ins.sync_info
            w = list(si.on_wait) if si is not None and si.on_wait else []
            if len(w) > 1:
                for k, extra in enumerate(w[:-1]):
                    out.append(mybir.InstNoOp(
                        name=f"{ins.name}_sw{k}", engine=ins.engine,
                        sync_info=mybir.SyncInfo(on_wait=[extra], on_update=[]),
                        bass_nofuse=True))
                ins.sync_info = mybir.SyncInfo(
                    on_wait=[w[-1]], on_update=list(si.on_update))
                changed = True
            out.append(ins)
        if changed:
            blk.instructions = out

    _prog_cache["nc"] = nc
    return nc


def _im2col(arr, wrap):
    if wrap:
        ap = np.pad(arr, ((0, 0), (0, 0), (1, 1), (1, 1)), mode="wrap")
    else:
        ap = np.pad(arr, ((0, 0), (0, 0), (1, 1), (1, 1)))
    sl = [ap[:, :, di:di + H, dj:dj + W] for di in range(3) for dj in range(3)]
    st = np.stack(sl, axis=2)
    return st.transpose(0, 3, 4, 1, 2).reshape(arr.shape[0] * H * W, C * 9)


def _split(v):
    h = v.astype(BF16)
    l = (v - h.astype(np.float32)).astype(BF16)
    return h, l


def _run(inputs, trace=False):
    from concourse.bass_utils import run_bass_kernel_spmd

    x = np.ascontiguousarray(np.asarray(inputs["x"], dtype=np.float32))
    images = np.ascontiguousarray(np.asarray(inputs["images"], dtype=np.float32))
    t = int(np.asarray(inputs["t"]))
    mu = float(np.asarray(inputs["mu_sched"])[t])
    sigma = float(np.asarray(inputs["sigma_sched"])[t])
    a = mu / (sigma * sigma)
    inv2s2 = 1.0 / (2.0 * sigma * sigma)

    xq = _im2col(x, wrap=True)
    patches = _im2col(images, wrap=False)
    c_all = ((-mu * mu * inv2s2) * np.sum(patches * patches, axis=1)).astype(np.float32)
    pcent = patches.reshape(NP, C, 9)[:, :, 4]

    v = (a * xq).astype(np.float32)
    xh, xl = _split(v)
    xhf = xh.astype(np.float32)

    xa_base = np.zeros((KA, NQ), BF16)
    xa_base[1:28] = xh.T
    xa_base[28:55] = xh.T
    xa_base[55:82] = xl.T
    xa_base[82:109] = xl.T
    xa_base[109] = BF16(1.0)
    xa_base[110] = BF16(1.0)

    in_maps = []
    zs = []
    for cc in range(NCORES):
        lo = cc * PLOC
        pf = patches[lo:lo + PLOC].astype(np.float32)
        ph, pl = _split(pf)
        ch_, cl = _split(c_all[lo:lo + PLOC])
        patm = np.zeros((KA, PLOC), BF16)
        patm[0] = BF16(1.0)
        patm[1:28] = ph.T
        patm[28:55] = pl.T
        patm[55:82] = ph.T
        patm[82:109] = pl.T
        patm[109] = ch_
        patm[110] = cl

        sub = slice(lo, lo + PLOC, SUB_STRIDE)
        gsub = (xhf @ patches[sub].T.astype(np.float32)
                + c_all[sub][None, :]).max(axis=1)
        zrow = (-(gsub + MARGIN) + B_S / A_S).astype(BF16)
        zs.append(zrow.astype(np.float32))

        xa = xa_base.copy()
        xa[0] = zrow

        pcb = pcent[lo:lo + PLOC].astype(BF16)
        pws = []
        for k in range(4):
            pw = np.zeros((128, NCHUNK, 128), BF16)
            for chnk in range(NCHUNK):
                pw[:, chnk, 32 * k:32 * k + 3] = pcb[chnk * 128:(chnk + 1) * 128]
                pw[:, chnk, 32 * k + 3] = BF16(1.0)
            pws.append(pw)
        m = {"patm": patm, "xa": xa}
        for k in range(4):
            m[f"pw{k}"] = pws[k]
        in_maps.append(m)

    nc = _build_program()
    res = run_bass_kernel_spmd(nc, in_maps, core_ids=list(range(NCORES)),
                               trace=trace)

    # host merge: core weights = e^{g + z_c - B/A}; rescale to min z frame
    z = np.stack(zs)                                   # [8, NQ]
    part = np.stack([r["out"] for r in res.results])   # [8, 4, NQ]
    Z = z.min(axis=0)
    fac = np.exp((Z[None, :] - z).astype(np.float64))  # [8, NQ] <= 1
    sum_w = (part[:, 3].astype(np.float64) * fac).sum(axis=0)
    wc = (part[:, 0:3].astype(np.float64) * fac[:, None, :]).sum(axis=0)

    xcT = x.reshape(B, C, H * W).transpose(1, 0, 2).reshape(C, NQ)
    out_q = (mu * wc / sum_w[None, :] - xcT) / (sigma * sigma)
    out = out_q.reshape(C, B, H, W).transpose(1, 0, 2, 3).astype(np.float32)
    return out, res


def kernel(**inputs) -> np.ndarray:
    out, _ = _run(inputs, trace=False)
    return out
